# revision 23
# baseline (speedup 1.0000x reference)
"""CLIP attention (B=32, S=577, D=1024, H=16) on 8 Trainium2 NeuronCores.

Sharding: data-parallel over batch — 4 images per core. All layout
transforms (x transpose, weight transpose/retile, bias retile, final
output transpose) happen on the host; the device computes entirely in a
transposed [feature, token] layout so no on-chip transposes are needed.

Device pipeline per image (per core):
  1. Q/K projections (mapping out[e,n] = wT.T @ xT) -> QT/KT [1024, 578]
  2. V projection in natural token layout (out[n,e] = xT.T @ wvT),
     scattered into per-head 65-column groups whose last column is 1.0
     (so the attention-value matmul also produces the softmax row sums)
  3. Per head: scoresT[k,q] = KT_h.T @ QT_h (softmax scale pre-folded
     into wq on host), pT = exp(scoresT) on ScalarE (no max subtraction:
     |scores| <= ~7 for this distribution, exp is safe in fp32),
     out_aug[65,q] = V_aug.T @ pT accumulated over k-chunks -> rows 0-63
     are the unnormalized output, row 64 the softmax denominator.
  4. Batched reciprocal of all 16 heads' denominators, then one K=16
     selector-matmul per feature chunk broadcasts 1/den across the two
     heads' 64-partition groups and VectorE multiplies it in.
  5. O projection back over heads -> finalT [1024, 578] -> DRAM.

Schedule: the PE is the bottleneck engine, so the kernel software-
pipelines across images to keep it fed. During image i's attention the
Q/K projection matmuls of image i+1 are woven in at (head, k-chunk)
granularity — they fill the PE while ScalarE exponentiates — and the
phase between attentions interleaves image i's denominator broadcast +
O projection with image i+1's V projection.

Matmul inputs use bfloat16 (~6e-3 rel err, 1 cycle/row PE rate at any
moving-dim size; f32r HIGH mode tripped the hardware power throttle to
a 50% duty cycle during attention). Accumulation stays fp32 in PSUM.
All weights are cached in SBUF once at kernel start (bf16: 8 MB).
"""

import numpy as np

B, S, D, H, DH = 32, 577, 1024, 16, 64
SCALE = DH ** -0.5
N_CORES = 8
BPC = B // N_CORES  # images per core
NT = BPC * S  # tokens per core
NDC = D // 128  # 8 partition chunks of the feature dim
# k-chunks of the sequence dim (stationary side of the AV matmul)
KCH = [(i * 128, min(128, S - i * 128)) for i in range((S + 127) // 128)]
SP = S + 1  # token axis padded 577 -> 578 (pad column zeroed on chip)
# token blocks for all [*, SP] matmuls: max moving dim is 512
TB = [(0, 512), (512, 66)]

_CACHE = {}


def _build():
    import concourse.mybir as mybir
    import concourse.tile as tile
    from concourse import bacc
    from contextlib import ExitStack

    f32 = mybir.dt.float32
    bf16 = mybir.dt.bfloat16
    EXP = mybir.ActivationFunctionType.Exp

    nc = bacc.Bacc()
    xT = nc.dram_tensor("xT", [NDC, 128, NT], bf16, kind="ExternalInput")
    wq = nc.dram_tensor("wq", [NDC, 128, D], bf16, kind="ExternalInput")
    wk = nc.dram_tensor("wk", [NDC, 128, D], bf16, kind="ExternalInput")
    wo = nc.dram_tensor("wo", [NDC, 128, D], bf16, kind="ExternalInput")
    wv = nc.dram_tensor("wv", [2, NDC, 128, 512], bf16, kind="ExternalInput")
    qb = nc.dram_tensor("qb", [128, NDC], f32, kind="ExternalInput")
    kb = nc.dram_tensor("kb", [128, NDC], f32, kind="ExternalInput")
    ob = nc.dram_tensor("ob", [128, NDC], f32, kind="ExternalInput")
    # per-head-scattered v bias [128, 16*65], col h*65+64 = 1.0
    vbb = nc.dram_tensor("vbb", [128, H * 65], f32, kind="ExternalInput")
    # selector for denominator broadcast, split in head halves so each
    # half's chain can run as soon as its 8 heads finish:
    # sel[hb, k, ch*128+m] = (hb*8 + k == 2*ch + m//64)
    sel = nc.dram_tensor("sel", [2, 8, D], bf16, kind="ExternalInput")
    outT = nc.dram_tensor("outT", [NDC, 128, NT], f32, kind="ExternalOutput")

    with ExitStack() as ctx:
        tc = ctx.enter_context(tile.TileContext(nc))
        const = ctx.enter_context(tc.tile_pool(name="const", bufs=1))
        xt_p = ctx.enter_context(tc.tile_pool(name="xt", bufs=10))
        qt_p = ctx.enter_context(tc.tile_pool(name="qt", bufs=17))
        kt_p = ctx.enter_context(tc.tile_pool(name="kt", bufs=17))
        vt_p = ctx.enter_context(tc.tile_pool(name="vt", bufs=6))
        pt_p = ctx.enter_context(tc.tile_pool(name="pt", bufs=12))
        ot_p = ctx.enter_context(tc.tile_pool(name="ot", bufs=9))
        ft_p = ctx.enter_context(tc.tile_pool(name="ft", bufs=3))
        dn_p = ctx.enter_context(tc.tile_pool(name="dn", bufs=2))
        # PSUM (8 banks): scores 2x2-bank, AV accum 2x1, projections 2x1
        ps2_p = ctx.enter_context(tc.tile_pool(name="ps2", bufs=2, space="PSUM"))
        av_p = ctx.enter_context(tc.tile_pool(name="av", bufs=2, space="PSUM"))
        pj_p = ctx.enter_context(tc.tile_pool(name="pj", bufs=2, space="PSUM"))

        def ps2_tile(p, n):
            return ps2_p.tile([p, n], f32, tag="ps2", name="ps2",
                              padded_shape=[128, 1024])

        def av_tile(p, n):
            return av_p.tile([p, n], f32, tag="av", name="av",
                             padded_shape=[128, 512])

        def pj_tile(p, n):
            return pj_p.tile([p, n], f32, tag="pj", name="pj",
                             padded_shape=[128, 512])

        vbb_t = const.tile([128, H * 65], f32, tag="vbb", name="vbb")
        qb_t = const.tile([128, NDC], f32, tag="qb", name="qb")
        kb_t = const.tile([128, NDC], f32, tag="kb", name="kb")
        ob_t = const.tile([128, NDC], f32, tag="ob", name="ob")
        for t, src in ((vbb_t, vbb), (qb_t, qb), (kb_t, kb), (ob_t, ob)):
            nc.sync.dma_start(out=t, in_=src[:, :])
        sel_t = []
        for hb in range(2):
            t = const.tile([8, D], bf16, tag=f"sel{hb}", name="sel")
            nc.sync.dma_start(out=t, in_=sel[hb, :, :])
            sel_t.append(t)
        vbb3 = vbb_t.rearrange("p (h u) -> p h u", u=65)

        def load_xt(img):
            t0 = img * S
            xt = []
            for dc in range(NDC):
                t = xt_p.tile([128, SP], bf16, tag="xt", name="xt")
                nc.sync.dma_start(out=t[:, 0:S], in_=xT[dc, :, t0:t0 + S])
                nc.gpsimd.memset(t[:, S:SP], 0.0)
                xt.append(t)
            return xt

        # x of image 0 before the bulk weight DMA: the first projection
        # block only needs xt0 + wq[0], so the PE can start early.
        # Remaining weight DMAs are issued lazily (interleaved with the
        # prologue blocks) to keep the DMA-semaphore waits tight.
        xt0 = load_xt(0)

        wq_t, wk_t, wo_t = [], [], []
        wv_t = {}

        def dma_w(name, wdram, dst, ec):
            t = const.tile([128, D], bf16, tag=f"{name}{ec}", name=name)
            nc.sync.dma_start(out=t, in_=wdram[ec, :, :])
            dst.append(t)

        def dma_wv(eb, dc):
            t = const.tile([128, 512], bf16, tag=f"wv{eb}_{dc}", name="wv")
            nc.sync.dma_start(out=t, in_=wv[eb, dc, :, :])
            wv_t[(eb, dc)] = t

        for ec in range(NDC):
            dma_w("wq", wq, wq_t, ec)

        def qk_proj_steps(xt, qkt):
            """Generator: yields after every dc step (2 matmuls) so the
            attention loop can weave these into PE bubbles."""
            for wcache, bias_t, dstl, pool, nm in (
                    (wq_t, qb_t, qkt["q"], qt_p, "qt"),
                    (wk_t, kb_t, qkt["k"], kt_p, "kt")):
                for ec in range(NDC):
                    w_t = wcache[ec]
                    dst = pool.tile([128, SP], bf16, tag=nm, name=nm)
                    ps0 = pj_tile(128, TB[0][1])
                    ps1 = pj_tile(128, TB[1][1])
                    for dc in range(NDC):
                        lhs = w_t[:, dc * 128:(dc + 1) * 128]
                        nc.tensor.matmul(
                            ps0, lhs, xt[dc][:, TB[0][0]:TB[0][0] + TB[0][1]],
                            start=(dc == 0), stop=(dc == NDC - 1))
                        nc.tensor.matmul(
                            ps1, lhs, xt[dc][:, TB[1][0]:TB[1][0] + TB[1][1]],
                            start=(dc == 0), stop=(dc == NDC - 1))
                        yield
                    nc.vector.tensor_scalar_add(
                        dst[:, TB[0][0]:TB[0][0] + TB[0][1]], ps0,
                        bias_t[:, ec:ec + 1])
                    nc.vector.tensor_scalar_add(
                        dst[:, TB[1][0]:TB[1][0] + TB[1][1]], ps1,
                        bias_t[:, ec:ec + 1])
                    dstl.append(dst)
                    yield

        def v_proj_chunk(xt, vt, kc, ps=None):
            """One k-chunk of the V projection (16 matmuls + scatter).
            ps picks the PSUM pool: the first chunk after an attention
            phase uses pj to avoid a WAR stall on the last head's
            AV-copy drain in the av pool."""
            k0, kn = KCH[kc]
            ps = ps or av_tile
            psv = [ps(kn, 512), ps(kn, 512)]
            for dc in range(NDC):
                lhs = xt[dc][:, k0:k0 + kn]
                for eb in range(2):
                    nc.tensor.matmul(
                        psv[eb], lhs, wv_t[(eb, dc)],
                        start=(dc == 0), stop=(dc == NDC - 1))
            dst3 = vt[kc].rearrange("p (h u) -> p h u", u=65)
            for eb in range(2):
                nc.vector.tensor_add(
                    dst3[:kn, eb * 8:(eb + 1) * 8, 0:64],
                    psv[eb].rearrange("p (h u) -> p h u", u=64),
                    vbb3[:kn, eb * 8:(eb + 1) * 8, 0:64],
                )
            nc.vector.tensor_copy(dst3[:kn, :, 64:65], vbb3[:kn, :, 64:65])

        # ---------------- prologue: image 0 projections, with the rest
        # of the weight DMAs issued between blocks ----------------
        qkt = {"q": [], "k": []}
        g0 = qk_proj_steps(xt0, qkt)
        for _ in range(2 * 9):  # two q blocks
            next(g0)
        for ec in range(NDC):
            dma_w("wk", wk, wk_t, ec)
        for _ in range(6 * 9):  # rest of the q blocks
            next(g0)
        for eb in range(2):
            for dc in range(NDC):
                dma_wv(eb, dc)
        for _ in range(4 * 9):  # four k blocks
            next(g0)
        for ec in range(NDC):
            dma_w("wo", wo, wo_t, ec)
        for _ in g0:
            pass
        vt = [vt_p.tile([128, H * 65], bf16, tag="vt", name="vt")
              for _ in range(len(KCH))]
        for kc in range(len(KCH)):
            v_proj_chunk(xt0, vt, kc)

        for img in range(BPC):
            qt, kt = qkt["q"], qkt["k"]
            nxt = img + 1 < BPC
            if nxt:
                xt_next = load_xt(img + 1)
                qkt_next = {"q": [], "k": []}
                gen = qk_proj_steps(xt_next, qkt_next)
            else:
                gen = iter(())

            def pull(n):
                for _ in range(n):
                    if next(gen, "end") == "end":
                        return

            ot = [ot_p.tile([128, SP], bf16, tag="ot", name="ot")
                  for _ in range(NDC)]
            # head h's denominator -> partition (h//4)*32, col block h%4
            den_st = dn_p.tile([128, 4 * SP], f32, tag="den_st", name="den_st")

            # ---- attention: single-head pipeline, AV one head behind ----
            pts = {}
            psa = {}

            def emit_sc(h, kc):
                ch, p0 = h // 2, (h % 2) * 64
                k0, kn = KCH[kc]
                lhsk = kt[ch][p0:p0 + 64, k0:k0 + kn]
                pss = ps2_tile(kn, SP)
                for q0, qn in TB:
                    nc.tensor.matmul(
                        pss[:, q0:q0 + qn], lhsk, qt[ch][p0:p0 + 64, q0:q0 + qn],
                        start=True, stop=True)
                pt = pt_p.tile([kn, SP], bf16, tag="pt", name="pt")
                nc.scalar.activation(pt, pss, EXP)
                pts[(h, kc)] = pt

            def emit_av(h, kc):
                k0, kn = KCH[kc]
                if kc == 0:
                    psa[h] = [av_tile(65, TB[0][1]), av_tile(65, TB[1][1])]
                lhsv = vt[kc][:kn, h * 65:(h + 1) * 65]
                for qi, (q0, qn) in enumerate(TB):
                    nc.tensor.matmul(
                        psa[h][qi], lhsv, pts[(h, kc)][:kn, q0:q0 + qn],
                        start=(kc == 0), stop=(kc == len(KCH) - 1))
                pts.pop((h, kc))

            def finish_av(h):
                ch, p0 = h // 2, (h % 2) * 64
                p4 = (h // 4) * 32
                c4 = (h % 4) * SP
                for qi, (q0, qn) in enumerate(TB):
                    nc.vector.tensor_copy(
                        ot[ch][p0:p0 + 64, q0:q0 + qn], psa[h][qi][0:64, :qn])
                    nc.vector.tensor_copy(
                        den_st[p4:p4 + 1, c4 + q0:c4 + q0 + qn],
                        psa[h][qi][64:65, :qn])
                psa.pop(h)

            # denominator chain per 8-head half: gather the finished
            # half's dens mid-attention so bcast(0..3) can weave into the
            # second half of the attention phase
            den_rr = [None, None]

            def den_half(hb):
                den_t = dn_p.tile([8, SP], f32, tag=f"den{hb}", name="den")
                nc.sync.dma_start(
                    out=den_t[:, :],
                    in_=den_st[hb * 64:hb * 64 + 64:32, :].rearrange(
                        "p (b s) -> p b s", s=SP))
                den_rf = dn_p.tile([8, SP], f32, tag=f"den_rf{hb}",
                                   name="den_rf")
                nc.vector.reciprocal(den_rf, den_t)
                rr = dn_p.tile([8, SP], bf16, tag=f"den_rr{hb}",
                               name="den_rr")
                nc.vector.tensor_copy(rr, den_rf)
                den_rr[hb] = rr

            def bcast(ch):
                hb = ch // 4
                for q0, qn in TB:
                    psb = pj_tile(128, qn)
                    nc.tensor.matmul(
                        psb, sel_t[hb][:, ch * 128:(ch + 1) * 128],
                        den_rr[hb][:, q0:q0 + qn], start=True, stop=True)
                    nc.vector.tensor_mul(
                        ot[ch][:, q0:q0 + qn], ot[ch][:, q0:q0 + qn], psb)

            for h in range(H):
                for kc in range(len(KCH)):
                    emit_sc(h, kc)
                    if h > 0:
                        emit_av(h - 1, kc)
                    pull(2)
                if h > 0:
                    finish_av(h - 1)
                    if h == 8:
                        den_half(0)  # heads 0-7 all finished
                    elif 11 <= h <= 14:
                        bcast(h - 11)
            for kc in range(len(KCH)):
                emit_av(H - 1, kc)
                pull(4)
            pull(1 << 20)  # drain any leftover projection steps
            finish_av(H - 1)
            den_half(1)

            if nxt:
                vt_next = [vt_p.tile([128, H * 65], bf16, tag="vt", name="vt")
                           for _ in range(len(KCH))]
            else:
                vt_next = None

            def o_proj(ec):
                w_t = wo_t[ec]
                ft = ft_p.tile([128, SP], f32, tag="ft", name="ft")
                ps0 = pj_tile(128, TB[0][1])
                ps1 = pj_tile(128, TB[1][1])
                for dc in range(NDC):
                    lhs = w_t[:, dc * 128:(dc + 1) * 128]
                    nc.tensor.matmul(
                        ps0, lhs, ot[dc][:, TB[0][0]:TB[0][0] + TB[0][1]],
                        start=(dc == 0), stop=(dc == NDC - 1))
                    nc.tensor.matmul(
                        ps1, lhs, ot[dc][:, TB[1][0]:TB[1][0] + TB[1][1]],
                        start=(dc == 0), stop=(dc == NDC - 1))
                nc.vector.tensor_scalar_add(
                    ft[:, TB[0][0]:TB[0][0] + TB[0][1]], ps0,
                    ob_t[:, ec:ec + 1])
                nc.vector.tensor_scalar_add(
                    ft[:, TB[1][0]:TB[1][0] + TB[1][1]], ps1,
                    ob_t[:, ec:ec + 1])
                nc.sync.dma_start(
                    out=outT[ec, :, img * S:img * S + S], in_=ft[:, 0:S])

            # ---- interleave img's bcast+O-proj with img+1's V proj so
            # the PE keeps running while the den chain completes ----
            # every bcast (and its DVE multiply) must land before ANY o_proj:
            # each o_proj block contracts over all 8 normalized ot chunks.
            # bcast(0..3) already ran inside the attention loop; V-proj
            # chunks of the next image cover the half-2 den chain here.
            if nxt:
                v_proj_chunk(xt_next, vt_next, 0, ps=pj_tile)
                v_proj_chunk(xt_next, vt_next, 1)
                bcast(4)
                bcast(5)
                v_proj_chunk(xt_next, vt_next, 2)
                bcast(6)
                bcast(7)
                v_proj_chunk(xt_next, vt_next, 3)
                o_proj(0)
                o_proj(1)
                v_proj_chunk(xt_next, vt_next, 4)
                for ec in range(2, NDC):
                    o_proj(ec)
                qkt = qkt_next
                vt = vt_next
            else:
                for ch in range(4, NDC):
                    bcast(ch)
                for ec in range(NDC):
                    o_proj(ec)

    nc.compile()
    return nc


def _get_nc():
    if "nc" not in _CACHE:
        _CACHE["nc"] = _build()
    return _CACHE["nc"]


def _host_prep(hidden_states, q_w, q_b, k_w, k_b, v_w, v_b, o_w, o_b):
    import ml_dtypes

    bf16 = ml_dtypes.bfloat16
    x = np.ascontiguousarray(np.asarray(hidden_states, dtype=np.float32))
    qw = np.asarray(q_w, np.float32) * SCALE
    qbv = np.asarray(q_b, np.float32) * SCALE
    kw = np.asarray(k_w, np.float32)
    kbv = np.asarray(k_b, np.float32)
    vw = np.asarray(v_w, np.float32)
    vbv = np.asarray(v_b, np.float32)
    ow = np.asarray(o_w, np.float32)
    obv = np.asarray(o_b, np.float32)

    def wT_retile_ec(w):
        # [ec, p, dc*128+j] = w.T[dc*128+p, ec*128+j]
        wt = w.T.reshape(NDC, 128, NDC, 128)  # [dc, p, ec, j]
        return np.ascontiguousarray(
            wt.transpose(2, 1, 0, 3).reshape(NDC, 128, D)).astype(bf16)

    def wT_retile_v(w):
        # [eb, dc, p, j] = w.T[dc*128+p, eb*512+j]
        wt = w.T.reshape(NDC, 128, 2, 512)  # [dc, p, eb, j]
        return np.ascontiguousarray(
            wt.transpose(2, 0, 1, 3).reshape(2, NDC, 128, 512)).astype(bf16)

    def b_retile(b):
        return np.ascontiguousarray(b.reshape(NDC, 128).T)

    wq_r = wT_retile_ec(qw)
    wk_r = wT_retile_ec(kw)
    wo_r = wT_retile_ec(ow)
    wv_r = wT_retile_v(vw)
    qb_r = b_retile(qbv)
    kb_r = b_retile(kbv)
    ob_r = b_retile(obv)
    vbb = np.empty((H, 65), np.float32)
    vbb[:, :64] = vbv.reshape(H, 64)
    vbb[:, 64] = 1.0
    vbb_r = np.ascontiguousarray(
        np.broadcast_to(vbb.reshape(-1), (128, H * 65)))
    sel_r = np.zeros((2, 8, D), bf16)
    for m in range(D):
        h = m // 64
        sel_r[h // 8, h % 8, m] = 1.0

    in_maps = []
    for c in range(N_CORES):
        xc = x[c * BPC:(c + 1) * BPC].reshape(NT, D)
        xTc = np.ascontiguousarray(xc.T).reshape(NDC, 128, NT).astype(bf16)
        in_maps.append(dict(
            xT=xTc, wq=wq_r, wk=wk_r, wv=wv_r, wo=wo_r,
            qb=qb_r, kb=kb_r, ob=ob_r, vbb=vbb_r, sel=sel_r,
        ))
    return in_maps


def kernel(hidden_states, q_w, q_b, k_w, k_b, v_w, v_b, o_w, o_b, **run_kwargs):
    from concourse.bass_utils import run_bass_kernel_spmd

    nc = _get_nc()
    in_maps = _host_prep(
        hidden_states, q_w, q_b, k_w, k_b, v_w, v_b, o_w, o_b)
    res = run_bass_kernel_spmd(
        nc, in_maps, core_ids=list(range(N_CORES)), **run_kwargs)
    outs = []
    for c in range(N_CORES):
        yT = res.results[c]["outT"].reshape(D, NT)
        outs.append(np.ascontiguousarray(yT.T).reshape(BPC, S, D))
    full = np.concatenate(outs, axis=0)
    if run_kwargs:
        return full, res
    return full


# revision 25
# speedup vs baseline: 1.0018x; 1.0018x over previous
"""CLIP attention (B=32, S=577, D=1024, H=16) on 8 Trainium2 NeuronCores.

Sharding: data-parallel over batch — 4 images per core. All layout
transforms (x transpose, weight transpose/retile, bias retile, final
output transpose) happen on the host; the device computes entirely in a
transposed [feature, token] layout so no on-chip transposes are needed.

Device pipeline per image (per core):
  1. Q/K projections (mapping out[e,n] = wT.T @ xT) -> QT/KT [1024, 578]
  2. V projection in natural token layout (out[n,e] = xT.T @ wvT),
     scattered into per-head 65-column groups whose last column is 1.0
     (so the attention-value matmul also produces the softmax row sums)
  3. Per head: scoresT[k,q] = KT_h.T @ QT_h (softmax scale pre-folded
     into wq on host), pT = exp(scoresT) on ScalarE (no max subtraction:
     |scores| <= ~7 for this distribution, exp is safe in fp32),
     out_aug[65,q] = V_aug.T @ pT accumulated over k-chunks -> rows 0-63
     are the unnormalized output, row 64 the softmax denominator.
  4. Batched reciprocal of all 16 heads' denominators, then one K=16
     selector-matmul per feature chunk broadcasts 1/den across the two
     heads' 64-partition groups and VectorE multiplies it in.
  5. O projection back over heads -> finalT [1024, 578] -> DRAM.

Schedule: the PE is the bottleneck engine, so the kernel software-
pipelines across images to keep it fed. During image i's attention the
Q/K projection matmuls of image i+1 are woven in at (head, k-chunk)
granularity — they fill the PE while ScalarE exponentiates — and the
phase between attentions interleaves image i's denominator broadcast +
O projection with image i+1's V projection.

Matmul inputs use bfloat16 (~6e-3 rel err, 1 cycle/row PE rate at any
moving-dim size; f32r HIGH mode tripped the hardware power throttle to
a 50% duty cycle during attention). Accumulation stays fp32 in PSUM.
All weights are cached in SBUF once at kernel start (bf16: 8 MB).
"""

import numpy as np

B, S, D, H, DH = 32, 577, 1024, 16, 64
SCALE = DH ** -0.5
N_CORES = 8
BPC = B // N_CORES  # images per core
NT = BPC * S  # tokens per core
NDC = D // 128  # 8 partition chunks of the feature dim
# k-chunks of the sequence dim (stationary side of the AV matmul)
KCH = [(i * 128, min(128, S - i * 128)) for i in range((S + 127) // 128)]
SP = S + 1  # token axis padded 577 -> 578 (pad column zeroed on chip)
# token blocks for all [*, SP] matmuls: max moving dim is 512
TB = [(0, 512), (512, 66)]

_CACHE = {}


def _build():
    import concourse.mybir as mybir
    import concourse.tile as tile
    from concourse import bacc
    from contextlib import ExitStack

    f32 = mybir.dt.float32
    bf16 = mybir.dt.bfloat16
    EXP = mybir.ActivationFunctionType.Exp

    nc = bacc.Bacc()
    xT = nc.dram_tensor("xT", [NDC, 128, NT], bf16, kind="ExternalInput")
    wq = nc.dram_tensor("wq", [NDC, 128, D], bf16, kind="ExternalInput")
    wk = nc.dram_tensor("wk", [NDC, 128, D], bf16, kind="ExternalInput")
    wo = nc.dram_tensor("wo", [NDC, 128, D], bf16, kind="ExternalInput")
    wv = nc.dram_tensor("wv", [2, NDC, 128, 512], bf16, kind="ExternalInput")
    qb = nc.dram_tensor("qb", [128, NDC], f32, kind="ExternalInput")
    kb = nc.dram_tensor("kb", [128, NDC], f32, kind="ExternalInput")
    ob = nc.dram_tensor("ob", [128, NDC], f32, kind="ExternalInput")
    # per-head-scattered v bias [128, 16*65], col h*65+64 = 1.0
    vbb = nc.dram_tensor("vbb", [128, H * 65], f32, kind="ExternalInput")
    # selector for denominator broadcast, split in head halves so each
    # half's chain can run as soon as its 8 heads finish:
    # sel[hb, k, ch*128+m] = (hb*8 + k == 2*ch + m//64)
    sel = nc.dram_tensor("sel", [2, 8, D], bf16, kind="ExternalInput")
    outT = nc.dram_tensor("outT", [NDC, 128, NT], f32, kind="ExternalOutput")

    with ExitStack() as ctx:
        tc = ctx.enter_context(tile.TileContext(nc))
        const = ctx.enter_context(tc.tile_pool(name="const", bufs=1))
        xt_p = ctx.enter_context(tc.tile_pool(name="xt", bufs=10))
        qt_p = ctx.enter_context(tc.tile_pool(name="qt", bufs=17))
        kt_p = ctx.enter_context(tc.tile_pool(name="kt", bufs=17))
        vt_p = ctx.enter_context(tc.tile_pool(name="vt", bufs=6))
        pt_p = ctx.enter_context(tc.tile_pool(name="pt", bufs=12))
        ot_p = ctx.enter_context(tc.tile_pool(name="ot", bufs=9))
        ft_p = ctx.enter_context(tc.tile_pool(name="ft", bufs=3))
        dn_p = ctx.enter_context(tc.tile_pool(name="dn", bufs=2))
        # PSUM (8 banks): scores 2x2-bank, AV accum 2x1, projections 2x1
        ps2_p = ctx.enter_context(tc.tile_pool(name="ps2", bufs=2, space="PSUM"))
        av_p = ctx.enter_context(tc.tile_pool(name="av", bufs=2, space="PSUM"))
        pj_p = ctx.enter_context(tc.tile_pool(name="pj", bufs=2, space="PSUM"))

        def ps2_tile(p, n):
            return ps2_p.tile([p, n], f32, tag="ps2", name="ps2",
                              padded_shape=[128, 1024])

        def av_tile(p, n):
            return av_p.tile([p, n], f32, tag="av", name="av",
                             padded_shape=[128, 512])

        def pj_tile(p, n):
            return pj_p.tile([p, n], f32, tag="pj", name="pj",
                             padded_shape=[128, 512])

        vbb_t = const.tile([128, H * 65], f32, tag="vbb", name="vbb")
        qb_t = const.tile([128, NDC], f32, tag="qb", name="qb")
        kb_t = const.tile([128, NDC], f32, tag="kb", name="kb")
        ob_t = const.tile([128, NDC], f32, tag="ob", name="ob")
        for t, src in ((vbb_t, vbb), (qb_t, qb), (kb_t, kb), (ob_t, ob)):
            nc.sync.dma_start(out=t, in_=src[:, :])
        sel_t = []
        for hb in range(2):
            t = const.tile([8, D], bf16, tag=f"sel{hb}", name="sel")
            nc.sync.dma_start(out=t, in_=sel[hb, :, :])
            sel_t.append(t)
        vbb3 = vbb_t.rearrange("p (h u) -> p h u", u=65)

        def load_xt(img):
            t0 = img * S
            xt = []
            for dc in range(NDC):
                t = xt_p.tile([128, SP], bf16, tag="xt", name="xt")
                nc.sync.dma_start(out=t[:, 0:S], in_=xT[dc, :, t0:t0 + S])
                nc.gpsimd.memset(t[:, S:SP], 0.0)
                xt.append(t)
            return xt

        # x of image 0 before the bulk weight DMA: the first projection
        # block only needs xt0 + wq[0], so the PE can start early.
        # Remaining weight DMAs are issued lazily (interleaved with the
        # prologue blocks) to keep the DMA-semaphore waits tight.
        xt0 = load_xt(0)

        wq_t, wk_t, wo_t = [], [], []
        wv_t = {}

        def dma_w(name, wdram, dst, ec):
            t = const.tile([128, D], bf16, tag=f"{name}{ec}", name=name)
            nc.sync.dma_start(out=t, in_=wdram[ec, :, :])
            dst.append(t)

        def dma_wv(eb, dc):
            t = const.tile([128, 512], bf16, tag=f"wv{eb}_{dc}", name="wv")
            nc.sync.dma_start(out=t, in_=wv[eb, dc, :, :])
            wv_t[(eb, dc)] = t

        for ec in range(NDC):
            dma_w("wq", wq, wq_t, ec)

        def qk_proj_steps(xt, qkt):
            """Generator: yields after every dc step (2 matmuls) so the
            attention loop can weave these into PE bubbles."""
            for wcache, bias_t, dstl, pool, nm in (
                    (wq_t, qb_t, qkt["q"], qt_p, "qt"),
                    (wk_t, kb_t, qkt["k"], kt_p, "kt")):
                for ec in range(NDC):
                    w_t = wcache[ec]
                    dst = pool.tile([128, SP], bf16, tag=nm, name=nm)
                    ps0 = pj_tile(128, TB[0][1])
                    ps1 = pj_tile(128, TB[1][1])
                    for dc in range(NDC):
                        lhs = w_t[:, dc * 128:(dc + 1) * 128]
                        nc.tensor.matmul(
                            ps0, lhs, xt[dc][:, TB[0][0]:TB[0][0] + TB[0][1]],
                            start=(dc == 0), stop=(dc == NDC - 1))
                        nc.tensor.matmul(
                            ps1, lhs, xt[dc][:, TB[1][0]:TB[1][0] + TB[1][1]],
                            start=(dc == 0), stop=(dc == NDC - 1))
                        yield
                    nc.vector.tensor_scalar_add(
                        dst[:, TB[0][0]:TB[0][0] + TB[0][1]], ps0,
                        bias_t[:, ec:ec + 1])
                    nc.vector.tensor_scalar_add(
                        dst[:, TB[1][0]:TB[1][0] + TB[1][1]], ps1,
                        bias_t[:, ec:ec + 1])
                    dstl.append(dst)
                    yield

        def v_proj_chunk(xt, vt, kc, ps=None):
            """One k-chunk of the V projection (16 matmuls + scatter).
            ps picks the PSUM pool: the first chunk after an attention
            phase uses pj to avoid a WAR stall on the last head's
            AV-copy drain in the av pool."""
            k0, kn = KCH[kc]
            ps = ps or av_tile
            psv = [ps(kn, 512), ps(kn, 512)]
            for dc in range(NDC):
                lhs = xt[dc][:, k0:k0 + kn]
                for eb in range(2):
                    nc.tensor.matmul(
                        psv[eb], lhs, wv_t[(eb, dc)],
                        start=(dc == 0), stop=(dc == NDC - 1))
            dst3 = vt[kc].rearrange("p (h u) -> p h u", u=65)
            for eb in range(2):
                nc.vector.tensor_add(
                    dst3[:kn, eb * 8:(eb + 1) * 8, 0:64],
                    psv[eb].rearrange("p (h u) -> p h u", u=64),
                    vbb3[:kn, eb * 8:(eb + 1) * 8, 0:64],
                )
            nc.vector.tensor_copy(dst3[:kn, :, 64:65], vbb3[:kn, :, 64:65])

        # ---------------- prologue: image 0 projections, with the rest
        # of the weight DMAs issued between blocks ----------------
        qkt = {"q": [], "k": []}
        g0 = qk_proj_steps(xt0, qkt)
        for _ in range(2 * 9):  # two q blocks
            next(g0)
        for ec in range(NDC):
            dma_w("wk", wk, wk_t, ec)
        for _ in range(6 * 9):  # rest of the q blocks
            next(g0)
        for eb in range(2):
            for dc in range(NDC):
                dma_wv(eb, dc)
        for _ in range(4 * 9):  # four k blocks
            next(g0)
        for ec in range(NDC):
            dma_w("wo", wo, wo_t, ec)
        for _ in g0:
            pass
        vt = [vt_p.tile([128, H * 65], bf16, tag="vt", name="vt")
              for _ in range(len(KCH))]
        for kc in range(len(KCH)):
            v_proj_chunk(xt0, vt, kc)

        for img in range(BPC):
            qt, kt = qkt["q"], qkt["k"]
            nxt = img + 1 < BPC
            if nxt:
                xt_next = load_xt(img + 1)
                qkt_next = {"q": [], "k": []}
                gen = qk_proj_steps(xt_next, qkt_next)
            else:
                gen = iter(())

            def pull(n):
                for _ in range(n):
                    if next(gen, "end") == "end":
                        return

            ot = [ot_p.tile([128, SP], bf16, tag="ot", name="ot")
                  for _ in range(NDC)]
            # head h's denominator -> partition (h//4)*32, col block h%4
            den_st = dn_p.tile([128, 4 * SP], f32, tag="den_st", name="den_st")

            # ---- attention: single-head pipeline, AV one head behind ----
            pts = {}
            psa = {}

            def emit_sc(h, kc):
                ch, p0 = h // 2, (h % 2) * 64
                k0, kn = KCH[kc]
                lhsk = kt[ch][p0:p0 + 64, k0:k0 + kn]
                pss = ps2_tile(kn, SP)
                for q0, qn in TB:
                    nc.tensor.matmul(
                        pss[:, q0:q0 + qn], lhsk, qt[ch][p0:p0 + 64, q0:q0 + qn],
                        start=True, stop=True)
                pt = pt_p.tile([kn, SP], bf16, tag="pt", name="pt")
                nc.scalar.activation(pt, pss, EXP)
                pts[(h, kc)] = pt

            def emit_av(h, kc):
                k0, kn = KCH[kc]
                if kc == 0:
                    psa[h] = [av_tile(65, TB[0][1]), av_tile(65, TB[1][1])]
                lhsv = vt[kc][:kn, h * 65:(h + 1) * 65]
                for qi, (q0, qn) in enumerate(TB):
                    nc.tensor.matmul(
                        psa[h][qi], lhsv, pts[(h, kc)][:kn, q0:q0 + qn],
                        start=(kc == 0), stop=(kc == len(KCH) - 1))
                pts.pop((h, kc))

            def finish_av(h):
                ch, p0 = h // 2, (h % 2) * 64
                p4 = (h // 4) * 32
                c4 = (h % 4) * SP
                for qi, (q0, qn) in enumerate(TB):
                    nc.vector.tensor_copy(
                        ot[ch][p0:p0 + 64, q0:q0 + qn], psa[h][qi][0:64, :qn])
                    nc.vector.tensor_copy(
                        den_st[p4:p4 + 1, c4 + q0:c4 + q0 + qn],
                        psa[h][qi][64:65, :qn])
                psa.pop(h)

            # denominator chain per 8-head half: gather the finished
            # half's dens mid-attention so bcast(0..3) can weave into the
            # second half of the attention phase
            den_rr = [None, None]

            def den_half(hb):
                den_t = dn_p.tile([8, SP], f32, tag=f"den{hb}", name="den")
                nc.sync.dma_start(
                    out=den_t[:, :],
                    in_=den_st[hb * 64:hb * 64 + 64:32, :].rearrange(
                        "p (b s) -> p b s", s=SP))
                den_rf = dn_p.tile([8, SP], f32, tag=f"den_rf{hb}",
                                   name="den_rf")
                nc.vector.reciprocal(den_rf, den_t)
                rr = dn_p.tile([8, SP], bf16, tag=f"den_rr{hb}",
                               name="den_rr")
                nc.vector.tensor_copy(rr, den_rf)
                den_rr[hb] = rr

            def bcast(ch):
                hb = ch // 4
                for q0, qn in TB:
                    psb = pj_tile(128, qn)
                    nc.tensor.matmul(
                        psb, sel_t[hb][:, ch * 128:(ch + 1) * 128],
                        den_rr[hb][:, q0:q0 + qn], start=True, stop=True)
                    nc.vector.tensor_mul(
                        ot[ch][:, q0:q0 + qn], ot[ch][:, q0:q0 + qn], psb)

            for h in range(H):
                for kc in range(len(KCH)):
                    emit_sc(h, kc)
                    if h > 0:
                        emit_av(h - 1, kc)
                    pull(2)
                if h > 0:
                    finish_av(h - 1)
                    if h == 8:
                        den_half(0)  # heads 0-7 all finished; DMA+DVE only
            for kc in range(len(KCH)):
                emit_av(H - 1, kc)
                pull(4)
            pull(1 << 20)  # drain any leftover projection steps
            finish_av(H - 1)
            den_half(1)

            if nxt:
                vt_next = [vt_p.tile([128, H * 65], bf16, tag="vt", name="vt")
                           for _ in range(len(KCH))]
            else:
                vt_next = None

            def o_proj(ec):
                w_t = wo_t[ec]
                ft = ft_p.tile([128, SP], f32, tag="ft", name="ft")
                ps0 = pj_tile(128, TB[0][1])
                ps1 = pj_tile(128, TB[1][1])
                for dc in range(NDC):
                    lhs = w_t[:, dc * 128:(dc + 1) * 128]
                    nc.tensor.matmul(
                        ps0, lhs, ot[dc][:, TB[0][0]:TB[0][0] + TB[0][1]],
                        start=(dc == 0), stop=(dc == NDC - 1))
                    nc.tensor.matmul(
                        ps1, lhs, ot[dc][:, TB[1][0]:TB[1][0] + TB[1][1]],
                        start=(dc == 0), stop=(dc == NDC - 1))
                nc.vector.tensor_scalar_add(
                    ft[:, TB[0][0]:TB[0][0] + TB[0][1]], ps0,
                    ob_t[:, ec:ec + 1])
                nc.vector.tensor_scalar_add(
                    ft[:, TB[1][0]:TB[1][0] + TB[1][1]], ps1,
                    ob_t[:, ec:ec + 1])
                nc.sync.dma_start(
                    out=outT[ec, :, img * S:img * S + S], in_=ft[:, 0:S])

            # ---- interleave img's bcast+O-proj with img+1's V proj so
            # the PE keeps running while the den chain completes ----
            # every bcast (and its DVE multiply) must land before ANY o_proj:
            # each o_proj block contracts over all 8 normalized ot chunks.
            # The first V chunks borrow the scores (ps2) PSUM pool — its
            # last user, exp of head 15, drained during the attention tail,
            # so they start without waiting on av/pj pool drains.
            if nxt:
                v_proj_chunk(xt_next, vt_next, 0, ps=ps2_tile)
                bcast(0)
                bcast(1)
                bcast(2)
                bcast(3)
                v_proj_chunk(xt_next, vt_next, 1, ps=ps2_tile)
                bcast(4)
                bcast(5)
                v_proj_chunk(xt_next, vt_next, 2)
                bcast(6)
                bcast(7)
                v_proj_chunk(xt_next, vt_next, 3)
                o_proj(0)
                o_proj(1)
                v_proj_chunk(xt_next, vt_next, 4)
                for ec in range(2, NDC):
                    o_proj(ec)
                qkt = qkt_next
                vt = vt_next
            else:
                for ch in range(4):
                    bcast(ch)
                for ch in range(4, NDC):
                    bcast(ch)
                for ec in range(NDC):
                    o_proj(ec)

    nc.compile()
    return nc


def _get_nc():
    if "nc" not in _CACHE:
        _CACHE["nc"] = _build()
    return _CACHE["nc"]


def _host_prep(hidden_states, q_w, q_b, k_w, k_b, v_w, v_b, o_w, o_b):
    import ml_dtypes

    bf16 = ml_dtypes.bfloat16
    x = np.ascontiguousarray(np.asarray(hidden_states, dtype=np.float32))
    qw = np.asarray(q_w, np.float32) * SCALE
    qbv = np.asarray(q_b, np.float32) * SCALE
    kw = np.asarray(k_w, np.float32)
    kbv = np.asarray(k_b, np.float32)
    vw = np.asarray(v_w, np.float32)
    vbv = np.asarray(v_b, np.float32)
    ow = np.asarray(o_w, np.float32)
    obv = np.asarray(o_b, np.float32)

    def wT_retile_ec(w):
        # [ec, p, dc*128+j] = w.T[dc*128+p, ec*128+j]
        wt = w.T.reshape(NDC, 128, NDC, 128)  # [dc, p, ec, j]
        return np.ascontiguousarray(
            wt.transpose(2, 1, 0, 3).reshape(NDC, 128, D)).astype(bf16)

    def wT_retile_v(w):
        # [eb, dc, p, j] = w.T[dc*128+p, eb*512+j]
        wt = w.T.reshape(NDC, 128, 2, 512)  # [dc, p, eb, j]
        return np.ascontiguousarray(
            wt.transpose(2, 0, 1, 3).reshape(2, NDC, 128, 512)).astype(bf16)

    def b_retile(b):
        return np.ascontiguousarray(b.reshape(NDC, 128).T)

    wq_r = wT_retile_ec(qw)
    wk_r = wT_retile_ec(kw)
    wo_r = wT_retile_ec(ow)
    wv_r = wT_retile_v(vw)
    qb_r = b_retile(qbv)
    kb_r = b_retile(kbv)
    ob_r = b_retile(obv)
    vbb = np.empty((H, 65), np.float32)
    vbb[:, :64] = vbv.reshape(H, 64)
    vbb[:, 64] = 1.0
    vbb_r = np.ascontiguousarray(
        np.broadcast_to(vbb.reshape(-1), (128, H * 65)))
    sel_r = np.zeros((2, 8, D), bf16)
    for m in range(D):
        h = m // 64
        sel_r[h // 8, h % 8, m] = 1.0

    in_maps = []
    for c in range(N_CORES):
        xc = x[c * BPC:(c + 1) * BPC].reshape(NT, D)
        xTc = np.ascontiguousarray(xc.T).reshape(NDC, 128, NT).astype(bf16)
        in_maps.append(dict(
            xT=xTc, wq=wq_r, wk=wk_r, wv=wv_r, wo=wo_r,
            qb=qb_r, kb=kb_r, ob=ob_r, vbb=vbb_r, sel=sel_r,
        ))
    return in_maps


def kernel(hidden_states, q_w, q_b, k_w, k_b, v_w, v_b, o_w, o_b, **run_kwargs):
    from concourse.bass_utils import run_bass_kernel_spmd

    nc = _get_nc()
    in_maps = _host_prep(
        hidden_states, q_w, q_b, k_w, k_b, v_w, v_b, o_w, o_b)
    res = run_bass_kernel_spmd(
        nc, in_maps, core_ids=list(range(N_CORES)), **run_kwargs)
    outs = []
    for c in range(N_CORES):
        yT = res.results[c]["outT"].reshape(D, NT)
        outs.append(np.ascontiguousarray(yT.T).reshape(BPC, S, D))
    full = np.concatenate(outs, axis=0)
    if run_kwargs:
        return full, res
    return full
